# revision 34
# baseline (speedup 1.0000x reference)
# DeepSet Trainium2 kernel, v3.
#
# Events sorted by jet-count n on the host (one mid-size group leads to
# fill the pipeline, then large-to-small), round-robin sharded across 8
# cores into per-group slots of capacity cap_g (multiple of 256). Within
# a group every event has exactly n=g valid jets: masks, pair structures
# and aggregation counts are compile-time constants.
#
# Design (rates measured on HW via microbench):
#  - all-bf16 activations AND weights (precision study: rel err ~6e-3
#    worst-case vs the 2e-2 gate; measured 2.9e-3).
#  - pairs enumerated DIAGONAL-major: pairs (i, i+d) grouped by d, so the
#    pair-sum z_i + z_j is g-1 big contiguous tensor_tensor ops
#    (2 elem/cyc bf16 DVE path), no per-pair ops. The bz bias + relu ride
#    one tensor_scalar (2-4 elem/cyc).
#  - all four Sum/Sumsq reductions via PSUM-accumulating identity matmuls
#    on the PE (0.42 ns/row); Max via DVE 4-slot tensor_tensor folds.
#  - NO PE transposes: outputs leave feature-major; x-side ships
#    [sum|max|sumsq], y-side ships [sum0|sum1|max|q0|q1] with the PSUM
#    accumulator halves unfolded; the host transposes, folds halves and
#    derives mean/var.
#  - PSUM evacs on Act (relu+bias activation, ~1.1 ns/col); avoid
#    scalar_tensor_tensor (1 elem/cyc on HW despite the cost model).
#  - 3-phase software pipeline: each iteration emits the jets MLP of
#    group k+2, the pair layers of group k+1 (with group k+2's y1 stage
#    hoisted between them), and the aggregations of group k, so the
#    in-order PE/Act/DVE streams always hold independent ready work.
import math
from contextlib import ExitStack

import numpy as np

import concourse.bass as bass
import concourse.bacc as bacc
import concourse.tile as tile
import concourse.mybir as mybir

f32 = mybir.dt.float32
bf16 = mybir.dt.bfloat16
AF = mybir.ActivationFunctionType
ALU = mybir.AluOpType

H = 128
FJ = 16
CH = 1536          # PSUM chunk width (3 banks)
NEG = -3.0e38

# evac engine per layer: 'act' or 'dve'  (tunable)
EV_X1, EV_X2, EV_X3, EV_Z = "act", "act", "act", "dve"
EV_Y2, EV_Y3 = "act", "act"


def pairs_diag(g):
    return [(i, i + d) for d in range(1, g) for i in range(g - d)]


def build_program(groups):
    """groups: list of (g, cap) with cap a multiple of 256, cap <= 512."""
    JC = sum(g * cap for g, cap in groups)
    EC = sum(cap for _, cap in groups)

    nc = bacc.Bacc("TRN2", target_bir_lowering=False, debug=False)

    jets_d = nc.dram_tensor("jets", [FJ, JC], bf16, kind="ExternalInput")
    w1_d = nc.dram_tensor("w1", [FJ, H], bf16, kind="ExternalInput")
    w2_d = nc.dram_tensor("w2", [H, H], bf16, kind="ExternalInput")
    w3_d = nc.dram_tensor("w3", [H, H], bf16, kind="ExternalInput")
    wz_d = nc.dram_tensor("wz", [H, H], bf16, kind="ExternalInput")
    w4_d = nc.dram_tensor("w4", [H, H], bf16, kind="ExternalInput")
    w5_d = nc.dram_tensor("w5", [H, H], bf16, kind="ExternalInput")
    ident_d = nc.dram_tensor("ident", [H, H], bf16, kind="ExternalInput")
    # bias cols: 0..5 = b1, b2, b3, bz(t), b4, b5
    bv_d = nc.dram_tensor("bvec", [H, 8], f32, kind="ExternalInput")
    # feature-major outputs; y carries unfolded accumulator halves
    outx_d = nc.dram_tensor("outx", [H, 3 * EC], f32, kind="ExternalOutput")
    outy_d = nc.dram_tensor("outy", [H, 5 * EC], f32, kind="ExternalOutput")

    with tile.TileContext(nc) as tc, ExitStack() as ctx:
        consts = ctx.enter_context(tc.tile_pool(name="consts", bufs=1))
        jin = ctx.enter_context(tc.tile_pool(name="jin", bufs=2))
        xp = ctx.enter_context(tc.tile_pool(name="xp", bufs=2))
        xk = ctx.enter_context(tc.tile_pool(name="xk", bufs=3))
        pp = ctx.enter_context(tc.tile_pool(name="pp", bufs=5))
        mxp = ctx.enter_context(tc.tile_pool(name="mxp", bufs=2))
        aggp = ctx.enter_context(tc.tile_pool(name="aggp", bufs=2))
        scp = ctx.enter_context(tc.tile_pool(name="scp", bufs=1))
        mm = ctx.enter_context(tc.tile_pool(name="mm", bufs=2, space="PSUM"))
        acc = ctx.enter_context(tc.tile_pool(name="acc", bufs=2, space="PSUM"))

        def const_tile(name, dram, shape, dt):
            t = consts.tile(shape, dt, tag=name)
            nc.sync.dma_start(t[:], dram.ap())
            return t

        w1t = const_tile("w1", w1_d, [FJ, H], bf16)
        bv = const_tile("bv", bv_d, [H, 8], f32)
        # prefetch the first two groups' jets before the remaining consts
        _pre_jt = {}
        _off = 0
        for _gi, (_g, _cap) in enumerate(groups[:2]):
            _n = _g * _cap
            _t = jin.tile([FJ, _n], bf16, tag="jt")
            nc.sync.dma_start(_t[:], jets_d.ap()[:, _off : _off + _n])
            _pre_jt[_gi] = _t
            _off += _n
        w2t = const_tile("w2", w2_d, [H, H], bf16)
        w3t = const_tile("w3", w3_d, [H, H], bf16)
        wzt = const_tile("wz", wz_d, [H, H], bf16)
        w4t = const_tile("w4", w4_d, [H, H], bf16)
        w5t = const_tile("w5", w5_d, [H, H], bf16)
        idt = const_tile("id", ident_d, [H, H], bf16)

        def stt(out, in0, scalar, in1, op0, op1):
            nc.vector.scalar_tensor_tensor(out, in0, scalar, in1, op0, op1)

        def layer(dst, wt, src, bias_col, evac, ch=None):
            """One dense layer over full width of src; relu+bias evac
            unless bias_col is None (plain copy)."""
            ch = ch or CH
            width = dst.shape[-1]
            chunks = []
            for c0 in range(0, width, ch):
                w = min(ch, width - c0)
                ps = mm.tile([H, CH], f32, tag="mm")
                for s0 in range(0, w, 512):
                    sw = min(512, w - s0)
                    nc.tensor.matmul(ps[:, s0 : s0 + sw], wt[:],
                                     src[:, c0 + s0 : c0 + s0 + sw],
                                     start=True, stop=True)
                chunks.append((ps, c0, w))
            for ps, c0, w in chunks:
                o = dst[:, c0 : c0 + w]
                if bias_col is None:
                    if evac == "act":
                        nc.scalar.copy(o, ps[:, :w])
                    else:
                        nc.vector.tensor_scalar(o, ps[:, :w], 1.0, None,
                                                ALU.mult)
                elif evac == "act":
                    nc.scalar.activation(o, ps[:, :w], AF.Relu,
                                         bias=bv[:, bias_col : bias_col + 1])
                else:
                    nc.vector.tensor_scalar(o, ps[:, :w],
                                            bv[:, bias_col : bias_col + 1],
                                            0.0, ALU.add, ALU.max)

        def rr(ap, k):
            return ap.rearrange("p (k c) -> p k c", k=k)

        def sum_tree(src, m, cap, out, wtag):
            """Sum m cap-slices of src (bf16) -> out [H, cap] (f32 ap)."""
            if m == 1:
                nc.vector.tensor_scalar(out, src[:, 0:cap], 1.0, None,
                                        ALU.mult)
                return
            cur, coff = src, 0
            while m > 1:
                k2, odd = m // 2, m % 2
                last = k2 == 1
                if last and not odd:
                    nc.vector.tensor_tensor(
                        out, cur[:, coff : coff + cap],
                        cur[:, coff + cap : coff + 2 * cap], ALU.add)
                    return
                dst = mxp.tile([H, k2 * cap], bf16, tag=wtag)
                nc.vector.tensor_tensor(
                    rr(dst[:, 0 : k2 * cap], k2),
                    rr(cur[:, coff : coff + k2 * cap], k2),
                    rr(cur[:, coff + k2 * cap : coff + 2 * k2 * cap], k2),
                    ALU.add)
                if odd:
                    lastsl = cur[:, coff + 2 * k2 * cap : coff + m * cap]
                    if k2 == 1:
                        nc.vector.tensor_tensor(out, dst[:, 0:cap], lastsl,
                                                ALU.add)
                        return
                    nc.vector.tensor_tensor(dst[:, 0:cap], dst[:, 0:cap],
                                            lastsl, ALU.add)
                if k2 == 1:
                    nc.vector.tensor_scalar(out, dst[:, 0:cap], 1.0, None,
                                            ALU.mult)
                    return
                cur, coff, m = dst, 0, k2

        def max_tree(src, m, cap, out, wtag):
            """Max over m cap-slices via a 4-slot fold (small scratch)."""
            if m == 1:
                nc.vector.tensor_scalar(out, src[:, 0:cap], 1.0, None,
                                        ALU.mult)
                return
            K = min(4, m)
            t = mxp.tile([H, K * cap], bf16, tag=wtag)
            nc.vector.tensor_copy(t[:], src[:, 0 : K * cap])
            j = K
            while j < m:
                w = min(K, m - j) * cap
                nc.vector.tensor_tensor(t[:, 0:w], t[:, 0:w],
                                        src[:, j * cap : j * cap + w],
                                        ALU.max)
                j += K
            if K == 4:
                nc.vector.tensor_tensor(t[:, 0 : 2 * cap], t[:, 0 : 2 * cap],
                                        t[:, 2 * cap : 4 * cap], ALU.max)
            if K >= 2:
                nc.vector.tensor_tensor(out, t[:, 0:cap], t[:, cap : 2 * cap],
                                        ALU.max)
            else:
                nc.vector.tensor_scalar(out, t[:, 0:cap], 1.0, None, ALU.mult)
            if K == 3:
                nc.vector.tensor_tensor(out, out, t[:, 2 * cap : 3 * cap],
                                        ALU.max)

        # Preload the activation table (Relu/Copy set) during input DMA
        # so the first real evac doesn't eat the ACT_TABLE_LOAD stall.
        warm = consts.tile([H, 1], f32, tag="warm")
        nc.scalar.activation(warm[:], bv[:, 0:1], AF.Relu, bias=bv[:, 0:1])

        def sum_mms(src, width, nsl, cap, out, wide=False):
            """Sum nsl cap-slices of src via PSUM-accumulating identity
            matmuls (two-half accumulator for cap==256). With wide=True,
            out is [H, 2*cap] and receives both UNFOLDED halves (the host
            adds them); otherwise out is [H, cap], folded here."""
            twoh = cap == 256
            acc_w = 512 if twoh else cap
            a_t = acc.tile([H, acc_w], f32, tag="acc")
            n0 = 0
            while n0 < width:
                sl = n0 // cap
                if twoh:
                    nw = min(512, width - n0)
                    o0 = (sl % 2) * 256
                    last = sl + (nw + 255) // 256 - 1
                else:
                    o0 = 0
                    nw = min(512, cap, width - n0)
                    last = sl
                nc.tensor.matmul(a_t[:, o0 : o0 + nw], idt[:],
                                 src[:, n0 : n0 + nw],
                                 start=(n0 == 0), stop=(last == nsl - 1))
                n0 += nw
            if wide:
                if twoh and nsl >= 2:
                    nc.vector.tensor_scalar(out, a_t[:, 0 : 2 * cap], 1.0,
                                            None, ALU.mult)
                else:
                    nc.vector.tensor_scalar(out[:, 0:cap], a_t[:, 0:cap],
                                            1.0, None, ALU.mult)
                    nc.vector.memset(out[:, cap : 2 * cap], 0.0)
            else:
                nc.vector.tensor_scalar(out, a_t[:, 0:cap], 1.0, None,
                                        ALU.mult)
                if twoh and nsl >= 2:
                    nc.vector.tensor_tensor(out, out, a_t[:, cap : 2 * cap],
                                            ALU.add)

        def x_stack(st):
            """Jets-side MLP chain for one group."""
            g, cap, jets_off = st["g"], st["cap"], st["jets_off"]
            N = g * cap
            if st["gi"] in _pre_jt:
                jt = _pre_jt.pop(st["gi"])
            else:
                jt = jin.tile([FJ, N], bf16, tag="jt")
                nc.sync.dma_start(jt[:],
                                  jets_d.ap()[:, jets_off : jets_off + N])
            ch = st.get("ch")
            x1 = xp.tile([H, N], bf16, tag="x1")
            layer(x1, w1t, jt, 0, EV_X1, ch)
            x2 = xp.tile([H, N], bf16, tag="x2")
            layer(x2, w2t, x1, 1, EV_X2, ch)
            x = xk.tile([H, N], bf16, tag="x")
            layer(x, w3t, x2, 2, EV_X3, ch)
            z = xk.tile([H, N], bf16, tag="z")
            layer(z, wzt, x, None, EV_Z, ch)
            st["x"], st["z"] = x, z

        def y1stage(st):
            """y1 = relu(z_i + z_j + t), diagonal-major, in-place relu."""
            g, cap = st["g"], st["cap"]
            z = st["z"]
            PG = g * (g - 1) // 2
            M = PG * cap
            y1p = pp.tile([H, M], bf16, tag="pp")
            off = 0
            for d in range(1, g):
                w = (g - d) * cap
                nc.vector.tensor_tensor(y1p[:, off : off + w], z[:, 0:w],
                                        z[:, d * cap : d * cap + w], ALU.add)
                off += w
            for c0 in range(0, M, 2048):
                w = min(2048, M - c0)
                nc.vector.tensor_scalar(y1p[:, c0 : c0 + w],
                                        y1p[:, c0 : c0 + w], bv[:, 3:4], 0.0,
                                        ALU.add, ALU.max)
            st["y1"] = y1p

        def f_stage(st, st_next):
            """Pair MLP layers y2,y3 for group st; next group's y1 stage is
            emitted between them (its z exists) to keep the DVE fed."""
            g, cap = st["g"], st["cap"]
            PG = g * (g - 1) // 2
            M = PG * cap
            y2 = pp.tile([H, M], bf16, tag="pp")
            layer(y2, w4t, st["y1"], 4, EV_Y2)
            if st_next is not None:
                y1stage(st_next)
            y3 = pp.tile([H, M], bf16, tag="pp")
            layer(y3, w5t, y2, 5, EV_Y3)
            st["y3"] = y3

        def a_stage(st):
            """Aggregations + output. aggx = [sum|max|sumsq] (3 slices);
            aggy = [sum0|sum1|max|q0|q1] (5 slices, halves folded on the
            host). ysq squares y3 via the pool slot y3 occupied."""
            g, cap, ev3, ev5 = st["g"], st["cap"], st["ev3"], st["ev5"]
            x, y3 = st["x"], st["y3"]
            N = g * cap
            PG = g * (g - 1) // 2
            M = PG * cap
            aggx = aggp.tile([H, 3 * cap], f32, tag="aggx")
            aggy = aggp.tile([H, 5 * cap], f32, tag="aggy")

            xsq = xp.tile([H, N], bf16, tag="xsq")
            nc.vector.tensor_tensor(xsq[:], x[:], x[:], ALU.mult)
            sum_mms(x, N, g, cap, aggx[:, 0:cap])
            sum_mms(xsq, N, g, cap, aggx[:, 2 * cap : 3 * cap])
            max_tree(x, g, cap, aggx[:, cap : 2 * cap], "mx")
            max_tree(y3, PG, cap, aggy[:, 2 * cap : 3 * cap], "mx")
            sum_mms(y3, M, PG, cap, aggy[:, 0 : 2 * cap], wide=True)

            ysq = pp.tile([H, M], bf16, tag="pp")
            for c0 in range(0, M, 2048):
                w = min(2048, M - c0)
                nc.vector.tensor_tensor(ysq[:, c0 : c0 + w],
                                        y3[:, c0 : c0 + w],
                                        y3[:, c0 : c0 + w], ALU.mult)
            sum_mms(ysq, M, PG, cap, aggy[:, 3 * cap : 5 * cap], wide=True)

            nc.sync.dma_start(outx_d.ap()[:, ev3 : ev3 + 3 * cap], aggx[:])
            nc.sync.dma_start(outy_d.ap()[:, ev5 : ev5 + 5 * cap], aggy[:])

        # Software pipeline, 3-phase skew: per iteration emit the jets
        # MLP of group k+2, the pair layers of group k+1, and the
        # aggregations of group k, so PE/Act/DVE all have independent
        # ready work at every point of their in-order streams.
        sts = []
        jets_off = 0
        ev3 = 0
        ev5 = 0
        for g, cap in groups:
            assert cap <= 512
            sts.append({"g": g, "cap": cap, "jets_off": jets_off,
                        "ev3": ev3, "ev5": ev5, "gi": len(sts)})
            jets_off += g * cap
            ev3 += 3 * cap
            ev5 += 5 * cap
        n = len(sts)
        sts[0]["ch"] = 512
        x_stack(sts[0])
        y1stage(sts[0])
        if n > 1:
            x_stack(sts[1])
        f_stage(sts[0], sts[1] if n > 1 else None)
        for k in range(n):
            if k + 2 < n:
                x_stack(sts[k + 2])
            if k + 1 < n:
                f_stage(sts[k + 1], sts[k + 2] if k + 2 < n else None)
            a_stage(sts[k])

    nc.compile()
    return nc


# ---------------- host-side math ----------------

BN_EPS = 1e-3


def fold_params(inp):
    """Fold normalization + BN into per-layer (W, b) in float64."""
    mean_j = np.asarray(inp["mean_jets"], np.float64)
    std_j = np.asarray(inp["std_jets"], np.float64)
    w1f = np.asarray(inp["w1_first"], np.float64)
    w1r = np.asarray(inp["w1_rest"], np.float64)
    bn1 = np.asarray(inp["bn1"], np.float64)
    w2f = np.asarray(inp["w2_first"], np.float64)
    w2r = np.asarray(inp["w2_rest"], np.float64)
    bn2 = np.asarray(inp["bn2"], np.float64)

    def bn_sb(row):
        gm, bt, mu, vv = row[0], row[1], row[2], row[3]
        s = gm / np.sqrt(vv + BN_EPS)
        return s, bt - mu * s

    s11, t11 = bn_sb(bn1[0]); s12, t12 = bn_sb(bn1[1]); s13, t13 = bn_sb(bn1[2])
    s21, t21 = bn_sb(bn2[0]); s22, t22 = bn_sb(bn2[1]); s23, t23 = bn_sb(bn2[2])

    A = w1f / std_j[:, None]
    c = -(mean_j / std_j) @ w1f
    return dict(
        W1=A * s11[None, :], b1=c * s11 + t11,
        W2=w1r[0] * s12[None, :], b2=t12,
        W3=w1r[1] * s13[None, :], b3=t13,
        Wz=w2f * s21[None, :], bz=t21,
        W4=w2r[0] * s22[None, :], b4=t22,
        W5=w2r[1] * s23[None, :], b5=t23,
    )


# ---------------- full kernel entry point ----------------

N_CORES = 8

_cache = {}
_TRACE = [False]
_LAST_RESULT = [None]


def _get_program(groups_key):
    if groups_key not in _cache:
        _cache[groups_key] = build_program(list(groups_key))
    return _cache[groups_key]


def _plan(n):
    """groups = [(g, cap)]; slots[c][gi] = (padded idx array, real count)."""
    gs = []
    idx_by_g = {}
    for g in range(2, 11):
        idx = np.nonzero(n == g)[0]
        if len(idx):
            gs.append(g)
            idx_by_g[g] = idx
    stray = np.nonzero((n < 2) | (n > 10))[0]
    if len(stray):
        if not gs:
            gs.append(2)
            idx_by_g[2] = stray
        else:
            idx_by_g[gs[-1]] = np.concatenate([idx_by_g[gs[-1]], stray])
    groups = []
    slots = [[] for _ in range(N_CORES)]
    order = sorted(gs, reverse=True)
    if len(order) >= 3:
        lead = order[-3]
        order = [lead] + [g for g in order if g != lead]
    for g in order:
        idx = idx_by_g[g]
        per_core = [idx[c::N_CORES] for c in range(N_CORES)]
        mx = max(len(p) for p in per_core)
        cap = max(256, ((mx + 255) // 256) * 256)
        groups.append((g, cap))
        fill = idx[0]
        for c in range(N_CORES):
            p = per_core[c]
            pad = np.full(cap, p[0] if len(p) else fill, dtype=np.int64)
            pad[: len(p)] = p
            slots[c].append((pad, len(p)))
    return groups, slots


def _pack_jets(jets, groups, slots_c, np_dt):
    cols = []
    for (g, cap), (ids, _cnt) in zip(groups, slots_c):
        ev = jets[ids][:, :g, :]  # [cap, g, 16]
        cols.append(np.ascontiguousarray(ev.transpose(2, 1, 0)).reshape(
            FJ, g * cap))
    return np.concatenate(cols, axis=1).astype(np_dt, copy=False)


def kernel(**inputs):
    from concourse.bass_utils import run_bass_kernel_spmd

    jets = np.asarray(inputs["inputs_jets"], dtype=np.float32)
    B = jets.shape[0]
    mask = (jets != 0.0).any(-1)
    n = mask.sum(-1).astype(np.int64)
    if not np.array_equal(mask, np.arange(jets.shape[1])[None, :] < n[:, None]):
        order = np.argsort(~mask, axis=1, kind="stable")
        jets = np.take_along_axis(jets, order[:, :, None], axis=1)

    P = fold_params(inputs)
    groups, slots = _plan(n)
    nc = _get_program(tuple(groups))

    bf_np = mybir.dt.np(bf16)
    bvec = np.zeros((H, 8), np.float32)
    for i, k in enumerate(["b1", "b2", "b3", "bz", "b4", "b5"]):
        bvec[:, i] = P[k]
    common = {
        "w1": P["W1"].astype(bf_np), "w2": P["W2"].astype(bf_np),
        "w3": P["W3"].astype(bf_np), "wz": P["Wz"].astype(bf_np),
        "w4": P["W4"].astype(bf_np), "w5": P["W5"].astype(bf_np),
        "ident": np.eye(H, dtype=np.float32).astype(bf_np), "bvec": bvec,
    }
    in_maps = []
    for c in range(N_CORES):
        m = dict(common)
        m["jets"] = _pack_jets(jets, groups, slots[c], bf_np)
        in_maps.append(m)

    res = run_bass_kernel_spmd(nc, in_maps, core_ids=list(range(N_CORES)),
                               trace=_TRACE[0])
    _LAST_RESULT[0] = res

    agg_x = np.empty((B, 4 * H), np.float32)
    agg_y = np.empty((B, 4 * H), np.float32)
    for c in range(N_CORES):
        ox = res.results[c]["outx"]
        oy = res.results[c]["outy"]
        ev3 = 0
        ev5 = 0
        for (g, cap), (ids, cnt) in zip(groups, slots[c]):
            PG = g * (g - 1) // 2
            bx = ox[:, ev3 : ev3 + 3 * cap].reshape(H, 3, cap)
            s_ = bx[:, 0, :cnt].T
            m_ = bx[:, 1, :cnt].T
            q_ = bx[:, 2, :cnt].T
            mean = s_ / g
            agg_x[ids[:cnt]] = np.concatenate(
                [s_, m_, mean, q_ / g - mean * mean], 1)
            by = oy[:, ev5 : ev5 + 5 * cap].reshape(H, 5, cap)
            s_ = (by[:, 0, :cnt] + by[:, 1, :cnt]).T
            m_ = by[:, 2, :cnt].T
            q_ = (by[:, 3, :cnt] + by[:, 4, :cnt]).T
            mean = s_ / PG
            agg_y[ids[:cnt]] = np.concatenate(
                [s_, m_, mean, q_ / PG - mean * mean], 1)
            ev3 += 3 * cap
            ev5 += 5 * cap
    return agg_x, agg_y


# revision 35
# speedup vs baseline: 1.0063x; 1.0063x over previous
# DeepSet Trainium2 kernel, v3.
#
# Events sorted by jet-count n on the host (one mid-size group leads to
# fill the pipeline, then large-to-small), round-robin sharded across 8
# cores into per-group slots of capacity cap_g (multiple of 256). Within
# a group every event has exactly n=g valid jets: masks, pair structures
# and aggregation counts are compile-time constants.
#
# Design (rates measured on HW via microbench):
#  - all-bf16 activations AND weights (precision study: rel err ~6e-3
#    worst-case vs the 2e-2 gate; measured 2.9e-3).
#  - pairs enumerated DIAGONAL-major: pairs (i, i+d) grouped by d, so the
#    pair-sum z_i + z_j is g-1 big contiguous tensor_tensor ops
#    (2 elem/cyc bf16 DVE path), no per-pair ops. The bz bias + relu ride
#    one tensor_scalar (2-4 elem/cyc).
#  - all four Sum/Sumsq reductions via PSUM-accumulating identity matmuls
#    on the PE (0.42 ns/row); Max via DVE 4-slot tensor_tensor folds.
#  - NO PE transposes: outputs leave feature-major; x-side ships
#    [sum|max|sumsq], y-side ships [sum0|sum1|max|q0|q1] with the PSUM
#    accumulator halves unfolded; the host transposes, folds halves and
#    derives mean/var.
#  - PSUM evacs on Act (relu+bias activation, ~1.1 ns/col); avoid
#    scalar_tensor_tensor (1 elem/cyc on HW despite the cost model).
#  - 3-phase software pipeline: each iteration emits the jets MLP of
#    group k+2, the pair layers of group k+1 (with group k+2's y1 stage
#    hoisted between them), and the aggregations of group k, so the
#    in-order PE/Act/DVE streams always hold independent ready work.
import math
from contextlib import ExitStack

import numpy as np

import concourse.bass as bass
import concourse.bacc as bacc
import concourse.tile as tile
import concourse.mybir as mybir

f32 = mybir.dt.float32
bf16 = mybir.dt.bfloat16
AF = mybir.ActivationFunctionType
ALU = mybir.AluOpType

H = 128
FJ = 16
CH = 1536          # PSUM chunk width (3 banks)
NEG = -3.0e38

# evac engine per layer: 'act' or 'dve'  (tunable)
EV_X1, EV_X2, EV_X3, EV_Z = "act", "act", "act", "act"
EV_Y2, EV_Y3 = "act", "act"


def pairs_diag(g):
    return [(i, i + d) for d in range(1, g) for i in range(g - d)]


def build_program(groups):
    """groups: list of (g, cap) with cap a multiple of 256, cap <= 512."""
    JC = sum(g * cap for g, cap in groups)
    EC = sum(cap for _, cap in groups)

    nc = bacc.Bacc("TRN2", target_bir_lowering=False, debug=False)

    jets_d = nc.dram_tensor("jets", [FJ, JC], bf16, kind="ExternalInput")
    w1_d = nc.dram_tensor("w1", [FJ, H], bf16, kind="ExternalInput")
    w2_d = nc.dram_tensor("w2", [H, H], bf16, kind="ExternalInput")
    w3_d = nc.dram_tensor("w3", [H, H], bf16, kind="ExternalInput")
    wz_d = nc.dram_tensor("wz", [H, H], bf16, kind="ExternalInput")
    w4_d = nc.dram_tensor("w4", [H, H], bf16, kind="ExternalInput")
    w5_d = nc.dram_tensor("w5", [H, H], bf16, kind="ExternalInput")
    ident_d = nc.dram_tensor("ident", [H, H], bf16, kind="ExternalInput")
    # bias cols: 0..5 = b1, b2, b3, bz(t), b4, b5
    bv_d = nc.dram_tensor("bvec", [H, 8], f32, kind="ExternalInput")
    # feature-major outputs; y carries unfolded accumulator halves
    outx_d = nc.dram_tensor("outx", [H, 3 * EC], f32, kind="ExternalOutput")
    outy_d = nc.dram_tensor("outy", [H, 5 * EC], f32, kind="ExternalOutput")

    with tile.TileContext(nc) as tc, ExitStack() as ctx:
        consts = ctx.enter_context(tc.tile_pool(name="consts", bufs=1))
        jin = ctx.enter_context(tc.tile_pool(name="jin", bufs=2))
        xp = ctx.enter_context(tc.tile_pool(name="xp", bufs=2))
        xk = ctx.enter_context(tc.tile_pool(name="xk", bufs=3))
        pp = ctx.enter_context(tc.tile_pool(name="pp", bufs=5))
        mxp = ctx.enter_context(tc.tile_pool(name="mxp", bufs=2))
        aggp = ctx.enter_context(tc.tile_pool(name="aggp", bufs=2))
        scp = ctx.enter_context(tc.tile_pool(name="scp", bufs=1))
        mm = ctx.enter_context(tc.tile_pool(name="mm", bufs=2, space="PSUM"))
        acc = ctx.enter_context(tc.tile_pool(name="acc", bufs=2, space="PSUM"))

        def const_tile(name, dram, shape, dt):
            t = consts.tile(shape, dt, tag=name)
            nc.sync.dma_start(t[:], dram.ap())
            return t

        w1t = const_tile("w1", w1_d, [FJ, H], bf16)
        bv = const_tile("bv", bv_d, [H, 8], f32)
        # prefetch the first two groups' jets before the remaining consts
        _pre_jt = {}
        _off = 0
        for _gi, (_g, _cap) in enumerate(groups[:2]):
            _n = _g * _cap
            _t = jin.tile([FJ, _n], bf16, tag="jt")
            nc.sync.dma_start(_t[:], jets_d.ap()[:, _off : _off + _n])
            _pre_jt[_gi] = _t
            _off += _n
        w2t = const_tile("w2", w2_d, [H, H], bf16)
        w3t = const_tile("w3", w3_d, [H, H], bf16)
        wzt = const_tile("wz", wz_d, [H, H], bf16)
        w4t = const_tile("w4", w4_d, [H, H], bf16)
        w5t = const_tile("w5", w5_d, [H, H], bf16)
        idt = const_tile("id", ident_d, [H, H], bf16)

        def stt(out, in0, scalar, in1, op0, op1):
            nc.vector.scalar_tensor_tensor(out, in0, scalar, in1, op0, op1)

        def layer(dst, wt, src, bias_col, evac, ch=None):
            """One dense layer over full width of src; relu+bias evac
            unless bias_col is None (plain copy)."""
            ch = ch or CH
            width = dst.shape[-1]
            chunks = []
            for c0 in range(0, width, ch):
                w = min(ch, width - c0)
                ps = mm.tile([H, CH], f32, tag="mm")
                for s0 in range(0, w, 512):
                    sw = min(512, w - s0)
                    nc.tensor.matmul(ps[:, s0 : s0 + sw], wt[:],
                                     src[:, c0 + s0 : c0 + s0 + sw],
                                     start=True, stop=True)
                chunks.append((ps, c0, w))
            for ps, c0, w in chunks:
                o = dst[:, c0 : c0 + w]
                if bias_col is None:
                    if evac == "act":
                        nc.scalar.copy(o, ps[:, :w])
                    else:
                        nc.vector.tensor_scalar(o, ps[:, :w], 1.0, None,
                                                ALU.mult)
                elif evac == "act":
                    nc.scalar.activation(o, ps[:, :w], AF.Relu,
                                         bias=bv[:, bias_col : bias_col + 1])
                else:
                    nc.vector.tensor_scalar(o, ps[:, :w],
                                            bv[:, bias_col : bias_col + 1],
                                            0.0, ALU.add, ALU.max)

        def rr(ap, k):
            return ap.rearrange("p (k c) -> p k c", k=k)

        def sum_tree(src, m, cap, out, wtag):
            """Sum m cap-slices of src (bf16) -> out [H, cap] (f32 ap)."""
            if m == 1:
                nc.vector.tensor_scalar(out, src[:, 0:cap], 1.0, None,
                                        ALU.mult)
                return
            cur, coff = src, 0
            while m > 1:
                k2, odd = m // 2, m % 2
                last = k2 == 1
                if last and not odd:
                    nc.vector.tensor_tensor(
                        out, cur[:, coff : coff + cap],
                        cur[:, coff + cap : coff + 2 * cap], ALU.add)
                    return
                dst = mxp.tile([H, k2 * cap], bf16, tag=wtag)
                nc.vector.tensor_tensor(
                    rr(dst[:, 0 : k2 * cap], k2),
                    rr(cur[:, coff : coff + k2 * cap], k2),
                    rr(cur[:, coff + k2 * cap : coff + 2 * k2 * cap], k2),
                    ALU.add)
                if odd:
                    lastsl = cur[:, coff + 2 * k2 * cap : coff + m * cap]
                    if k2 == 1:
                        nc.vector.tensor_tensor(out, dst[:, 0:cap], lastsl,
                                                ALU.add)
                        return
                    nc.vector.tensor_tensor(dst[:, 0:cap], dst[:, 0:cap],
                                            lastsl, ALU.add)
                if k2 == 1:
                    nc.vector.tensor_scalar(out, dst[:, 0:cap], 1.0, None,
                                            ALU.mult)
                    return
                cur, coff, m = dst, 0, k2

        def max_tree(src, m, cap, out, wtag):
            """Max over m cap-slices via a 4-slot fold (small scratch)."""
            if m == 1:
                nc.vector.tensor_scalar(out, src[:, 0:cap], 1.0, None,
                                        ALU.mult)
                return
            K = min(4, m)
            t = mxp.tile([H, K * cap], bf16, tag=wtag)
            nc.vector.tensor_copy(t[:], src[:, 0 : K * cap])
            j = K
            while j < m:
                w = min(K, m - j) * cap
                nc.vector.tensor_tensor(t[:, 0:w], t[:, 0:w],
                                        src[:, j * cap : j * cap + w],
                                        ALU.max)
                j += K
            if K == 4:
                nc.vector.tensor_tensor(t[:, 0 : 2 * cap], t[:, 0 : 2 * cap],
                                        t[:, 2 * cap : 4 * cap], ALU.max)
            if K >= 2:
                nc.vector.tensor_tensor(out, t[:, 0:cap], t[:, cap : 2 * cap],
                                        ALU.max)
            else:
                nc.vector.tensor_scalar(out, t[:, 0:cap], 1.0, None, ALU.mult)
            if K == 3:
                nc.vector.tensor_tensor(out, out, t[:, 2 * cap : 3 * cap],
                                        ALU.max)

        # Preload the activation table (Relu/Copy set) during input DMA
        # so the first real evac doesn't eat the ACT_TABLE_LOAD stall.
        warm = consts.tile([H, 1], f32, tag="warm")
        nc.scalar.activation(warm[:], bv[:, 0:1], AF.Relu, bias=bv[:, 0:1])

        def sum_mms(src, width, nsl, cap, out, wide=False):
            """Sum nsl cap-slices of src via PSUM-accumulating identity
            matmuls (two-half accumulator for cap==256). With wide=True,
            out is [H, 2*cap] and receives both UNFOLDED halves (the host
            adds them); otherwise out is [H, cap], folded here."""
            twoh = cap == 256
            acc_w = 512 if twoh else cap
            a_t = acc.tile([H, acc_w], f32, tag="acc")
            n0 = 0
            while n0 < width:
                sl = n0 // cap
                if twoh:
                    nw = min(512, width - n0)
                    o0 = (sl % 2) * 256
                    last = sl + (nw + 255) // 256 - 1
                else:
                    o0 = 0
                    nw = min(512, cap, width - n0)
                    last = sl
                nc.tensor.matmul(a_t[:, o0 : o0 + nw], idt[:],
                                 src[:, n0 : n0 + nw],
                                 start=(n0 == 0), stop=(last == nsl - 1))
                n0 += nw
            if wide:
                if twoh and nsl >= 2:
                    nc.vector.tensor_scalar(out, a_t[:, 0 : 2 * cap], 1.0,
                                            None, ALU.mult)
                else:
                    nc.vector.tensor_scalar(out[:, 0:cap], a_t[:, 0:cap],
                                            1.0, None, ALU.mult)
                    nc.vector.memset(out[:, cap : 2 * cap], 0.0)
            else:
                nc.vector.tensor_scalar(out, a_t[:, 0:cap], 1.0, None,
                                        ALU.mult)
                if twoh and nsl >= 2:
                    nc.vector.tensor_tensor(out, out, a_t[:, cap : 2 * cap],
                                            ALU.add)

        def x_stack(st):
            """Jets-side MLP chain for one group."""
            g, cap, jets_off = st["g"], st["cap"], st["jets_off"]
            N = g * cap
            if st["gi"] in _pre_jt:
                jt = _pre_jt.pop(st["gi"])
            else:
                jt = jin.tile([FJ, N], bf16, tag="jt")
                nc.sync.dma_start(jt[:],
                                  jets_d.ap()[:, jets_off : jets_off + N])
            ch = st.get("ch")
            x1 = xp.tile([H, N], bf16, tag="x1")
            layer(x1, w1t, jt, 0, EV_X1, ch)
            x2 = xp.tile([H, N], bf16, tag="x2")
            layer(x2, w2t, x1, 1, EV_X2, ch)
            x = xk.tile([H, N], bf16, tag="x")
            layer(x, w3t, x2, 2, EV_X3, ch)
            z = xk.tile([H, N], bf16, tag="z")
            layer(z, wzt, x, None, EV_Z, ch)
            st["x"], st["z"] = x, z

        def y1stage(st):
            """y1 = relu(z_i + z_j + t), diagonal-major, in-place relu."""
            g, cap = st["g"], st["cap"]
            z = st["z"]
            PG = g * (g - 1) // 2
            M = PG * cap
            y1p = pp.tile([H, M], bf16, tag="pp")
            off = 0
            for d in range(1, g):
                w = (g - d) * cap
                nc.vector.tensor_tensor(y1p[:, off : off + w], z[:, 0:w],
                                        z[:, d * cap : d * cap + w], ALU.add)
                off += w
            for c0 in range(0, M, 2048):
                w = min(2048, M - c0)
                nc.vector.tensor_scalar(y1p[:, c0 : c0 + w],
                                        y1p[:, c0 : c0 + w], bv[:, 3:4], 0.0,
                                        ALU.add, ALU.max)
            st["y1"] = y1p

        def f_stage(st, st_next):
            """Pair MLP layers y2,y3 for group st; next group's y1 stage is
            emitted between them (its z exists) to keep the DVE fed."""
            g, cap = st["g"], st["cap"]
            PG = g * (g - 1) // 2
            M = PG * cap
            y2 = pp.tile([H, M], bf16, tag="pp")
            layer(y2, w4t, st["y1"], 4, EV_Y2)
            if st_next is not None:
                y1stage(st_next)
            y3 = pp.tile([H, M], bf16, tag="pp")
            layer(y3, w5t, y2, 5, EV_Y3)
            st["y3"] = y3

        def a_stage(st):
            """Aggregations + output. aggx = [sum|max|sumsq] (3 slices);
            aggy = [sum0|sum1|max|q0|q1] (5 slices, halves folded on the
            host). ysq squares y3 via the pool slot y3 occupied."""
            g, cap, ev3, ev5 = st["g"], st["cap"], st["ev3"], st["ev5"]
            x, y3 = st["x"], st["y3"]
            N = g * cap
            PG = g * (g - 1) // 2
            M = PG * cap
            aggx = aggp.tile([H, 3 * cap], f32, tag="aggx")
            aggy = aggp.tile([H, 5 * cap], f32, tag="aggy")

            xsq = xp.tile([H, N], bf16, tag="xsq")
            nc.vector.tensor_tensor(xsq[:], x[:], x[:], ALU.mult)
            sum_mms(x, N, g, cap, aggx[:, 0:cap])
            sum_mms(xsq, N, g, cap, aggx[:, 2 * cap : 3 * cap])
            max_tree(x, g, cap, aggx[:, cap : 2 * cap], "mx")
            max_tree(y3, PG, cap, aggy[:, 2 * cap : 3 * cap], "mx")
            sum_mms(y3, M, PG, cap, aggy[:, 0 : 2 * cap], wide=True)

            ysq = pp.tile([H, M], bf16, tag="pp")
            for c0 in range(0, M, 2048):
                w = min(2048, M - c0)
                nc.vector.tensor_tensor(ysq[:, c0 : c0 + w],
                                        y3[:, c0 : c0 + w],
                                        y3[:, c0 : c0 + w], ALU.mult)
            sum_mms(ysq, M, PG, cap, aggy[:, 3 * cap : 5 * cap], wide=True)

            nc.sync.dma_start(outx_d.ap()[:, ev3 : ev3 + 3 * cap], aggx[:])
            nc.sync.dma_start(outy_d.ap()[:, ev5 : ev5 + 5 * cap], aggy[:])

        # Software pipeline, 3-phase skew: per iteration emit the jets
        # MLP of group k+2, the pair layers of group k+1, and the
        # aggregations of group k, so PE/Act/DVE all have independent
        # ready work at every point of their in-order streams.
        sts = []
        jets_off = 0
        ev3 = 0
        ev5 = 0
        for g, cap in groups:
            assert cap <= 512
            sts.append({"g": g, "cap": cap, "jets_off": jets_off,
                        "ev3": ev3, "ev5": ev5, "gi": len(sts)})
            jets_off += g * cap
            ev3 += 3 * cap
            ev5 += 5 * cap
        n = len(sts)
        sts[0]["ch"] = 512
        x_stack(sts[0])
        y1stage(sts[0])
        if n > 1:
            x_stack(sts[1])
        f_stage(sts[0], sts[1] if n > 1 else None)
        for k in range(n):
            if k + 2 < n:
                x_stack(sts[k + 2])
            if k + 1 < n:
                f_stage(sts[k + 1], sts[k + 2] if k + 2 < n else None)
            a_stage(sts[k])

    nc.compile()
    return nc


# ---------------- host-side math ----------------

BN_EPS = 1e-3


def fold_params(inp):
    """Fold normalization + BN into per-layer (W, b) in float64."""
    mean_j = np.asarray(inp["mean_jets"], np.float64)
    std_j = np.asarray(inp["std_jets"], np.float64)
    w1f = np.asarray(inp["w1_first"], np.float64)
    w1r = np.asarray(inp["w1_rest"], np.float64)
    bn1 = np.asarray(inp["bn1"], np.float64)
    w2f = np.asarray(inp["w2_first"], np.float64)
    w2r = np.asarray(inp["w2_rest"], np.float64)
    bn2 = np.asarray(inp["bn2"], np.float64)

    def bn_sb(row):
        gm, bt, mu, vv = row[0], row[1], row[2], row[3]
        s = gm / np.sqrt(vv + BN_EPS)
        return s, bt - mu * s

    s11, t11 = bn_sb(bn1[0]); s12, t12 = bn_sb(bn1[1]); s13, t13 = bn_sb(bn1[2])
    s21, t21 = bn_sb(bn2[0]); s22, t22 = bn_sb(bn2[1]); s23, t23 = bn_sb(bn2[2])

    A = w1f / std_j[:, None]
    c = -(mean_j / std_j) @ w1f
    return dict(
        W1=A * s11[None, :], b1=c * s11 + t11,
        W2=w1r[0] * s12[None, :], b2=t12,
        W3=w1r[1] * s13[None, :], b3=t13,
        Wz=w2f * s21[None, :], bz=t21,
        W4=w2r[0] * s22[None, :], b4=t22,
        W5=w2r[1] * s23[None, :], b5=t23,
    )


# ---------------- full kernel entry point ----------------

N_CORES = 8

_cache = {}
_TRACE = [False]
_LAST_RESULT = [None]


def _get_program(groups_key):
    if groups_key not in _cache:
        _cache[groups_key] = build_program(list(groups_key))
    return _cache[groups_key]


def _plan(n):
    """groups = [(g, cap)]; slots[c][gi] = (padded idx array, real count)."""
    gs = []
    idx_by_g = {}
    for g in range(2, 11):
        idx = np.nonzero(n == g)[0]
        if len(idx):
            gs.append(g)
            idx_by_g[g] = idx
    stray = np.nonzero((n < 2) | (n > 10))[0]
    if len(stray):
        if not gs:
            gs.append(2)
            idx_by_g[2] = stray
        else:
            idx_by_g[gs[-1]] = np.concatenate([idx_by_g[gs[-1]], stray])
    groups = []
    slots = [[] for _ in range(N_CORES)]
    order = sorted(gs, reverse=True)
    if len(order) >= 3:
        lead = order[-3]
        order = [lead] + [g for g in order if g != lead]
    for g in order:
        idx = idx_by_g[g]
        per_core = [idx[c::N_CORES] for c in range(N_CORES)]
        mx = max(len(p) for p in per_core)
        cap = max(256, ((mx + 255) // 256) * 256)
        groups.append((g, cap))
        fill = idx[0]
        for c in range(N_CORES):
            p = per_core[c]
            pad = np.full(cap, p[0] if len(p) else fill, dtype=np.int64)
            pad[: len(p)] = p
            slots[c].append((pad, len(p)))
    return groups, slots


def _pack_jets(jets, groups, slots_c, np_dt):
    cols = []
    for (g, cap), (ids, _cnt) in zip(groups, slots_c):
        ev = jets[ids][:, :g, :]  # [cap, g, 16]
        cols.append(np.ascontiguousarray(ev.transpose(2, 1, 0)).reshape(
            FJ, g * cap))
    return np.concatenate(cols, axis=1).astype(np_dt, copy=False)


def kernel(**inputs):
    from concourse.bass_utils import run_bass_kernel_spmd

    jets = np.asarray(inputs["inputs_jets"], dtype=np.float32)
    B = jets.shape[0]
    mask = (jets != 0.0).any(-1)
    n = mask.sum(-1).astype(np.int64)
    if not np.array_equal(mask, np.arange(jets.shape[1])[None, :] < n[:, None]):
        order = np.argsort(~mask, axis=1, kind="stable")
        jets = np.take_along_axis(jets, order[:, :, None], axis=1)

    P = fold_params(inputs)
    groups, slots = _plan(n)
    nc = _get_program(tuple(groups))

    bf_np = mybir.dt.np(bf16)
    bvec = np.zeros((H, 8), np.float32)
    for i, k in enumerate(["b1", "b2", "b3", "bz", "b4", "b5"]):
        bvec[:, i] = P[k]
    common = {
        "w1": P["W1"].astype(bf_np), "w2": P["W2"].astype(bf_np),
        "w3": P["W3"].astype(bf_np), "wz": P["Wz"].astype(bf_np),
        "w4": P["W4"].astype(bf_np), "w5": P["W5"].astype(bf_np),
        "ident": np.eye(H, dtype=np.float32).astype(bf_np), "bvec": bvec,
    }
    in_maps = []
    for c in range(N_CORES):
        m = dict(common)
        m["jets"] = _pack_jets(jets, groups, slots[c], bf_np)
        in_maps.append(m)

    res = run_bass_kernel_spmd(nc, in_maps, core_ids=list(range(N_CORES)),
                               trace=_TRACE[0])
    _LAST_RESULT[0] = res

    agg_x = np.empty((B, 4 * H), np.float32)
    agg_y = np.empty((B, 4 * H), np.float32)
    for c in range(N_CORES):
        ox = res.results[c]["outx"]
        oy = res.results[c]["outy"]
        ev3 = 0
        ev5 = 0
        for (g, cap), (ids, cnt) in zip(groups, slots[c]):
            PG = g * (g - 1) // 2
            bx = ox[:, ev3 : ev3 + 3 * cap].reshape(H, 3, cap)
            s_ = bx[:, 0, :cnt].T
            m_ = bx[:, 1, :cnt].T
            q_ = bx[:, 2, :cnt].T
            mean = s_ / g
            agg_x[ids[:cnt]] = np.concatenate(
                [s_, m_, mean, q_ / g - mean * mean], 1)
            by = oy[:, ev5 : ev5 + 5 * cap].reshape(H, 5, cap)
            s_ = (by[:, 0, :cnt] + by[:, 1, :cnt]).T
            m_ = by[:, 2, :cnt].T
            q_ = (by[:, 3, :cnt] + by[:, 4, :cnt]).T
            mean = s_ / PG
            agg_y[ids[:cnt]] = np.concatenate(
                [s_, m_, mean, q_ / PG - mean * mean], 1)
            ev3 += 3 * cap
            ev5 += 5 * cap
    return agg_x, agg_y
